# revision 20
# baseline (speedup 1.0000x reference)
"""Trainium2 Bass kernel for nn_Decoder: fused single-step LSTM decoder.

Reference computation (per token t of batch b, state never advances):
    gates = x[b,t] @ W_ih.T + (h0[b] @ W_hh.T + b_ih + b_hh)     # [2048]
    i, f, g, o = sigmoid/sigmoid/tanh/sigmoid of gate quarters
    c = f * c0[b] + i * g
    h = o * tanh(c)
    out[b,t] = h @ fc_w.T + fc_b                                 # [513]

Sharding: data-parallel, batch 64 -> 8 batches per core on 8 NeuronCores.
Per-core layout strategy ("feature-major"):
  - x [8192 tok, 513] is DMA'd naturally into [128, 4, 576] tiles (cols
    513..575 pre-zeroed once), transposed on the PE (identity matmul,
    float32r at 1.5 cyc/row) to xT [d, tok]; the PSUM->SBUF drain copy
    rounds to float32r (TF32-like, full-rate matmul dtype on TRN2).
  - K=513 contraction = 4 full K=128 chunks + a K=1 matmul against the
    transposed window [512:576) whose row 0 is feature 512 (rows 1..63
    are the zeroed pad, never read).
  - gates are computed transposed: gatesT[g-chunk, tok] with W_ihT as
    the stationary operand. The per-batch bias const
    (h0 @ W_hh.T + b_ih + b_hh, precomputed on host) enters for free as
    the ScalarE activation per-partition bias AP.
  - LSTM cell math runs feature-major so c0 is a per-partition scalar;
    h lands directly in the [h, tok] layout the fc matmul needs as lhsT.
  - fc output [tok, 513] is computed token-major in two even N-halves
    (256+258, col 513 is zero-pad garbage), fc_b folded in via a K=1
    ones-row matmul. Output copies PSUM->SBUF on ScalarE, DMA per
    128-token subtile.
"""

from contextlib import ExitStack

import numpy as np

import concourse.bass as bass
import concourse.tile as tile
from concourse import bacc, mybir
from concourse.bass_utils import run_bass_kernel_spmd
from concourse.masks import make_identity

FP32 = mybir.dt.float32
FP32R = mybir.dt.float32r
AFT = mybir.ActivationFunctionType

N_CORES = 8
B, T, D = 64, 1024, 513
H = 512
G4 = 4 * H  # 2048
B_LOC = B // N_CORES  # 8 batches per core
TOK = B_LOC * T  # 8192 tokens per core
TT = 512  # tokens per tile (stays within one batch: T % TT == 0)
NT = TOK // TT  # 16 tiles
NM = TT // 128  # 4 token-subtiles of 128
DX = 576  # x tile column pad (cols 513..575 zeroed; window [512:576) puts
#           feature 512 at partition 0 of the 5th transposed chunk)
DPAD = 514  # fc output padded to even width (col 513 is zero garbage)
NSPLIT = [256, 258]  # fc N split, both even (fp32r needs even N)


def build_nc(reps=1):
    nc = bacc.Bacc("TRN2", target_bir_lowering=False, debug=False, num_devices=N_CORES)
    x = nc.dram_tensor("x", [TOK, D], FP32, kind="ExternalInput").ap()
    wih_t = nc.dram_tensor("wih_t", [H, G4], FP32, kind="ExternalInput").ap()
    wih_row = nc.dram_tensor("wih_row", [1, G4], FP32, kind="ExternalInput").ap()
    fc_rhs = nc.dram_tensor("fc_rhs", [H, DPAD], FP32, kind="ExternalInput").ap()
    fc_row = nc.dram_tensor("fc_row", [1, DPAD], FP32, kind="ExternalInput").ap()
    bct = nc.dram_tensor("bconst", [128, 16 * B_LOC], FP32, kind="ExternalInput").ap()
    c0t = nc.dram_tensor("c0t", [128, 4 * B_LOC], FP32, kind="ExternalInput").ap()
    out = nc.dram_tensor("out", [TOK, D], FP32, kind="ExternalOutput").ap()

    with tile.TileContext(nc) as tc, ExitStack() as ctx:
        const = ctx.enter_context(tc.tile_pool(name="const", bufs=1))
        stage = ctx.enter_context(tc.tile_pool(name="stage", bufs=2))
        xtp = ctx.enter_context(tc.tile_pool(name="xt", bufs=12))
        x4p = ctx.enter_context(tc.tile_pool(name="xt4", bufs=3))
        sigp = ctx.enter_context(tc.tile_pool(name="sig", bufs=8))
        tmpp = ctx.enter_context(tc.tile_pool(name="tmp", bufs=2))
        hp = ctx.enter_context(tc.tile_pool(name="h", bufs=8))
        outp = ctx.enter_context(tc.tile_pool(name="osb", bufs=2))
        # PSUM: 8 banks total: pst 2 + gates 2 + fc halves 2+2
        pstp = ctx.enter_context(tc.tile_pool(name="pst", bufs=2, space="PSUM"))
        gpp = ctx.enter_context(tc.tile_pool(name="gp", bufs=2, space="PSUM"))
        fmp = ctx.enter_context(tc.tile_pool(name="fm", bufs=2, space="PSUM"))
        fcp = ctx.enter_context(tc.tile_pool(name="fcc", bufs=2, space="PSUM"))

        dma_engines = [nc.sync, nc.scalar, nc.gpsimd]

        # ---- constants (one-time) ----
        ident = const.tile([128, 128], FP32, tag="ident")
        make_identity(nc, ident[:])

        # x slots, manually rotated so the zeroed tail columns persist
        xn_slots = [
            const.tile([128, NM, DX], FP32, tag=f"xns{i}", name=f"xns{i}")
            for i in range(3)
        ]
        for s in xn_slots:
            nc.vector.memset(s[:, :, D:DX], 0.0)

        def load_weight_chunk(k):
            tmp = stage.tile([128, G4], FP32, tag="wstage")
            dma_engines[k % 3].dma_start(tmp[:], wih_t[k * 128 : (k + 1) * 128, :])
            w = const.tile([128, G4], FP32R, tag=f"w{k}")
            nc.vector.tensor_copy(w[:], tmp[:])
            return w

        def load_fc_chunk(k):
            tmp = stage.tile([128, DPAD], FP32, tag="fstage")
            dma_engines[(k + 1) % 3].dma_start(
                tmp[:], fc_rhs[k * 128 : (k + 1) * 128, :]
            )
            w = const.tile([128, DPAD], FP32R, tag=f"f{k}")
            nc.vector.tensor_copy(w[:], tmp[:])
            return w

        gate_funcs = [AFT.Sigmoid, AFT.Sigmoid, AFT.Tanh, AFT.Sigmoid]

        def emit_transposes(tt):
            """DMA in x for tile tt and transpose to xT chunks (fp32r)."""
            ts = tt * TT
            xn = xn_slots[tt % 3]
            [nc.sync, nc.gpsimd][tt % 2].dma_start(
                xn[:, :, 0:D], x[ts : ts + TT, :].rearrange("(m p) d -> p m d", m=NM)
            )
            xt = []
            for k in range(4):
                ps = pstp.tile([128, TT], FP32, tag="pst")
                for m in range(NM):
                    nc.tensor.transpose(
                        ps[:, m * 128 : (m + 1) * 128],
                        xn[:, m, k * 128 : (k + 1) * 128],
                        ident[:],
                    )
                t = xtp.tile([128, TT], FP32R, tag="xt")
                nc.vector.tensor_copy(t[:], ps[:])
                xt.append(t)
            # 5th chunk: window [512:576) -> feature 512 lands at partition 0
            ps = pstp.tile([128, TT], FP32, tag="pst")
            for m in range(NM):
                nc.tensor.transpose(
                    ps[0:64, m * 128 : (m + 1) * 128],
                    xn[:, m, H : H + 64],
                    ident[:],
                )
            t4 = x4p.tile([1, TT], FP32R, tag="xt4")
            nc.vector.tensor_copy(t4[:], ps[0:1, :])
            xt.append(t4)
            return xt

        # ---- startup: interleave first two tiles with weight loads so the
        # PE has transpose work while weights share the serial HBM pipe ----
        xt_queue = [emit_transposes(0)]
        wsb = [load_weight_chunk(0), load_weight_chunk(1)]
        xt_queue.append(emit_transposes(1))
        wsb += [load_weight_chunk(2), load_weight_chunk(3)]

        wr_stage = stage.tile([1, G4], FP32, tag="wrstage", bufs=1)
        nc.sync.dma_start(wr_stage[:], wih_row)
        w4 = const.tile([1, G4], FP32R, tag="w4")
        nc.vector.tensor_copy(w4[:], wr_stage[:])
        wsb.append(w4)

        fcsb = [load_fc_chunk(k) for k in range(4)]
        fr_stage = stage.tile([1, DPAD], FP32, tag="frstage", bufs=1)
        nc.scalar.dma_start(fr_stage[:], fc_row)
        f4 = const.tile([1, DPAD], FP32R, tag="f4")
        nc.vector.tensor_copy(f4[:], fr_stage[:])
        fcsb.append(f4)

        ones_st = stage.tile([1, 128], FP32, tag="ones_st", bufs=1)
        nc.vector.memset(ones_st[:], 1.0)
        ones = const.tile([1, 128], FP32R, tag="ones")
        nc.vector.tensor_copy(ones[:], ones_st[:])

        # per-batch bias const and c0, feature-major columns
        bct_sb = const.tile([128, 16 * B_LOC], FP32, tag="bct")
        nc.gpsimd.dma_start(bct_sb[:], bct)
        c0_sb = const.tile([128, 4 * B_LOC], FP32, tag="c0")
        nc.gpsimd.dma_start(c0_sb[:], c0t)

        # ---- main loop over 16 token tiles of 512 ----
        # (optionally repeated `reps` times inside one NEFF for timing)
        rep_ctx = tc.For_i(0, reps, 1) if reps > 1 else None
        if rep_ctx is not None:
            rep_ctx.__enter__()
        for tt in range(NT):
            b = tt // (T // TT)
            ts = tt * TT
            xt = xt_queue.pop(0)

            # prefetch + transpose two tiles ahead while this tile computes
            # (reps>1 wraps around so each For_i iteration is steady-state)
            if reps > 1:
                xt_queue.append(emit_transposes((tt + 2) % NT))
            elif tt + 2 < NT:
                xt_queue.append(emit_transposes(tt + 2))

            # gates + LSTM cell, per h-chunk k
            hn = []
            for k in range(4):
                gs = []
                for gi in range(4):
                    c = gi * 4 + k  # g-chunk index in [0, 16)
                    ps = gpp.tile([128, TT], FP32, tag="gp")
                    for kd in range(4):
                        nc.tensor.matmul(
                            ps[:],
                            wsb[kd][:, c * 128 : (c + 1) * 128],
                            xt[kd][:],
                            start=(kd == 0),
                            stop=False,
                        )
                    nc.tensor.matmul(
                        ps[:],
                        wsb[4][:, c * 128 : (c + 1) * 128],
                        xt[4][:],
                        start=False,
                        stop=True,
                    )
                    s = sigp.tile([128, TT], FP32, tag="sig")
                    nc.scalar.activation(
                        s[:],
                        ps[:],
                        gate_funcs[gi],
                        bias=bct_sb[:, c * B_LOC + b : c * B_LOC + b + 1],
                    )
                    gs.append(s)
                i_s, f_s, g_s, o_s = gs
                t1 = tmpp.tile([128, TT], FP32, tag="t1")
                nc.vector.tensor_mul(t1[:], i_s[:], g_s[:])
                t2 = tmpp.tile([128, TT], FP32, tag="t2")
                nc.vector.tensor_scalar_mul(
                    t2[:], f_s[:], c0_sb[:, k * B_LOC + b : k * B_LOC + b + 1]
                )
                cc = tmpp.tile([128, TT], FP32, tag="cc")
                nc.vector.tensor_add(cc[:], t1[:], t2[:])
                th = tmpp.tile([128, TT], FP32, tag="th")
                nc.scalar.activation(th[:], cc[:], AFT.Tanh)
                h = hp.tile([128, TT], FP32R, tag="h")
                nc.vector.tensor_mul(h[:], o_s[:], th[:])
                hn.append(h)

            # fc: out[tok, 513] per 128-token subtile, N split 256 + 258
            osb = outp.tile([128, NM, DPAD], FP32, tag="osb")
            for m in range(NM):
                msl = slice(m * 128, (m + 1) * 128)
                lhs5 = [
                    hn[0][:, msl],
                    hn[1][:, msl],
                    hn[2][:, msl],
                    hn[3][:, msl],
                    ones[:],
                ]
                pm = fmp.tile([128, NSPLIT[0]], FP32, tag="fm")
                for kd in range(5):
                    nc.tensor.matmul(
                        pm[:],
                        lhs5[kd],
                        fcsb[kd][:, 0 : NSPLIT[0]],
                        start=(kd == 0),
                        stop=(kd == 4),
                    )
                pc = fcp.tile([128, NSPLIT[1]], FP32, tag="fcc")
                for kd in range(5):
                    nc.tensor.matmul(
                        pc[:],
                        lhs5[kd],
                        fcsb[kd][:, NSPLIT[0] : DPAD],
                        start=(kd == 0),
                        stop=(kd == 4),
                    )
                nc.scalar.copy(osb[:, m, 0 : NSPLIT[0]], pm[:])
                nc.scalar.copy(osb[:, m, NSPLIT[0] : DPAD], pc[:])
                [nc.gpsimd, nc.sync][(tt + m) % 2].dma_start(
                    out[ts + m * 128 : ts + (m + 1) * 128, :], osb[:, m, 0:D]
                )
        if rep_ctx is not None:
            rep_ctx.__exit__(None, None, None)

    nc.compile()
    return nc


_NC_CACHE = []


def get_nc():
    if not _NC_CACHE:
        _NC_CACHE.append(build_nc())
    return _NC_CACHE[0]


def make_in_maps(decoder_inputs, h0, c0, W_ih, W_hh, b_ih, b_hh, fc_w, fc_b):
    di = np.ascontiguousarray(np.asarray(decoder_inputs, dtype=np.float32))
    h0 = np.asarray(h0, dtype=np.float32)[0]  # [64, 512]
    c0 = np.asarray(c0, dtype=np.float32)[0]
    W_ih = np.asarray(W_ih, dtype=np.float32)
    W_hh = np.asarray(W_hh, dtype=np.float32)
    b_ih = np.asarray(b_ih, dtype=np.float32)
    b_hh = np.asarray(b_hh, dtype=np.float32)
    fc_w = np.asarray(fc_w, dtype=np.float32)
    fc_b = np.asarray(fc_b, dtype=np.float32)

    bc = h0 @ W_hh.T + b_ih + b_hh  # [64, 2048]
    wih_tp = np.ascontiguousarray(W_ih.T[0:512])  # [512, 2048]
    wih_row_a = np.ascontiguousarray(W_ih.T[512:513])  # [1, 2048]
    fc_pad = np.zeros((H, DPAD), dtype=np.float32)
    fc_pad[:, 0:D] = fc_w.T
    fc_row_a = np.zeros((1, DPAD), dtype=np.float32)
    fc_row_a[0, 0:D] = fc_b

    in_maps = []
    for core in range(N_CORES):
        bs = core * B_LOC
        xc = di[bs : bs + B_LOC].reshape(TOK, D)
        # bct[p, c*8+b] = bc[bs+b, c*128+p]
        bct = np.ascontiguousarray(
            bc[bs : bs + B_LOC]
            .reshape(B_LOC, 16, 128)
            .transpose(2, 1, 0)
            .reshape(128, -1)
        )
        c0c = np.ascontiguousarray(
            c0[bs : bs + B_LOC]
            .reshape(B_LOC, 4, 128)
            .transpose(2, 1, 0)
            .reshape(128, -1)
        )
        in_maps.append(
            {
                "x": xc,
                "wih_t": wih_tp,
                "wih_row": wih_row_a,
                "fc_rhs": fc_pad,
                "fc_row": fc_row_a,
                "bconst": bct,
                "c0t": c0c,
            }
        )
    return in_maps


def kernel(**inputs):
    in_maps = make_in_maps(**inputs)
    nc = get_nc()
    res = run_bass_kernel_spmd(nc, in_maps, core_ids=list(range(N_CORES)))
    out = np.concatenate([res.results[c]["out"] for c in range(N_CORES)], axis=0)
    return out.reshape(B, T, D)


# revision 22
# speedup vs baseline: 2.5704x; 2.5704x over previous
"""Trainium2 Bass kernel for nn_Decoder: fused single-step LSTM decoder.

Reference computation (per token t of batch b, state never advances):
    gates = x[b,t] @ W_ih.T + (h0[b] @ W_hh.T + b_ih + b_hh)     # [2048]
    i, f, g, o = sigmoid/sigmoid/tanh/sigmoid of gate quarters
    c = f * c0[b] + i * g
    h = o * tanh(c)
    out[b,t] = h @ fc_w.T + fc_b                                 # [513]

Sharding: data-parallel, batch 64 -> 8 batches per core on 8 NeuronCores.
Per-core layout strategy ("feature-major"):
  - x is cast to bf16 on the host and zero-padded to 640 columns; the
    kernel never loads it natively. Each 512-token tile is brought in as
    5 DMA xbar-transposes (DRAM [512 tok, 128 d] -> SBUF [128 d, 512]),
    so the transpose costs zero PE/DVE time. The 5th window [512:640)
    holds feature 512 at row 0 (rest zero padding).
  - gates are computed transposed in bf16: gatesT[g-chunk, tok] with
    host-cast bf16 W_ihT as the stationary operand; the K=513
    contraction is 4 full K=128 chunks + one K=1 matmul for feature
    512. The per-batch bias const (h0 @ W_hh.T + b_ih + b_hh,
    precomputed fp32 on host) enters for free as the ScalarE activation
    per-partition bias AP.
  - LSTM cell math runs feature-major in fp32 so c0 is a per-partition
    scalar; h (rounded to float32r) lands directly in the [h, tok]
    layout the fc matmul needs as lhsT.
  - fc runs in float32r (TF32-like) for accuracy: out[tok, 513]
    token-major in two even N-halves (256+258, col 513 zero-pad),
    fc_b folded in via a K=1 ones-row matmul. Output copies
    PSUM->SBUF on ScalarE, DMA out per 128-token subtile.
"""

from contextlib import ExitStack

import ml_dtypes
import numpy as np

import concourse.bass as bass
import concourse.tile as tile
from concourse import bacc, mybir
from concourse.bass_utils import run_bass_kernel_spmd

FP32 = mybir.dt.float32
FP32R = mybir.dt.float32r
BF16 = mybir.dt.bfloat16
AFT = mybir.ActivationFunctionType

N_CORES = 8
B, T, D = 64, 1024, 513
H = 512
G4 = 4 * H  # 2048
B_LOC = B // N_CORES  # 8 batches per core
TOK = B_LOC * T  # 8192 tokens per core
TT = 512  # tokens per tile (stays within one batch: T % TT == 0)
NT = TOK // TT  # 16 tiles
NM = TT // 128  # 4 token-subtiles of 128
DX = 640  # host-padded x width: 5 transpose windows of 128 (cols 513+ zero)
DPAD = 514  # fc output padded to even width (col 513 is zero garbage)
NSPLIT = [256, 258]  # fc N split, both even (fp32r needs even N)


def build_nc(reps=1):
    nc = bacc.Bacc("TRN2", target_bir_lowering=False, debug=False, num_devices=N_CORES)
    x = nc.dram_tensor("x", [TOK, DX], BF16, kind="ExternalInput").ap()
    wih_t = nc.dram_tensor("wih_t", [H, G4], BF16, kind="ExternalInput").ap()
    wih_row = nc.dram_tensor("wih_row", [1, G4], BF16, kind="ExternalInput").ap()
    fc_rhs = nc.dram_tensor("fc_rhs", [H, DPAD], FP32, kind="ExternalInput").ap()
    fc_row = nc.dram_tensor("fc_row", [1, DPAD], FP32, kind="ExternalInput").ap()
    bct = nc.dram_tensor("bconst", [128, 16 * B_LOC], FP32, kind="ExternalInput").ap()
    c0t = nc.dram_tensor("c0t", [128, 4 * B_LOC], FP32, kind="ExternalInput").ap()
    out = nc.dram_tensor("out", [TOK, D], FP32, kind="ExternalOutput").ap()

    with tile.TileContext(nc) as tc, ExitStack() as ctx:
        const = ctx.enter_context(tc.tile_pool(name="const", bufs=1))
        stage = ctx.enter_context(tc.tile_pool(name="stage", bufs=2))
        xtp = ctx.enter_context(tc.tile_pool(name="xt", bufs=15))
        sigp = ctx.enter_context(tc.tile_pool(name="sig", bufs=8))
        tmpp = ctx.enter_context(tc.tile_pool(name="tmp", bufs=2))
        hp = ctx.enter_context(tc.tile_pool(name="h", bufs=8))
        outp = ctx.enter_context(tc.tile_pool(name="osb", bufs=3))
        # PSUM: 8 banks: gates 4 + fc halves 2+2
        gpp = ctx.enter_context(tc.tile_pool(name="gp", bufs=4, space="PSUM"))
        fmp = ctx.enter_context(tc.tile_pool(name="fm", bufs=2, space="PSUM"))
        fcp = ctx.enter_context(tc.tile_pool(name="fcc", bufs=2, space="PSUM"))

        # ---- weights (one-time; bf16 direct, fc staged + rounded to fp32r)
        wsb = []
        for k in range(4):
            w = const.tile([128, G4], BF16, tag=f"w{k}", name=f"w{k}")
            nc.sync.dma_start(w[:], wih_t[k * 128 : (k + 1) * 128, :])
            wsb.append(w)
        w4 = const.tile([1, G4], BF16, tag="w4")
        nc.sync.dma_start(w4[:], wih_row)
        wsb.append(w4)

        def load_fc_chunk(k):
            tmp = stage.tile([128, DPAD], FP32, tag="fstage")
            nc.sync.dma_start(tmp[:], fc_rhs[k * 128 : (k + 1) * 128, :])
            w = const.tile([128, DPAD], FP32R, tag=f"f{k}", name=f"f{k}")
            nc.vector.tensor_copy(w[:], tmp[:])
            return w

        gate_funcs = [AFT.Sigmoid, AFT.Sigmoid, AFT.Tanh, AFT.Sigmoid]

        def emit_transposes(tt):
            """Bring in tile tt as 5 transposed bf16 chunks via DMA xbar."""
            ts = tt * TT
            xt = []
            for k in range(5):
                t = xtp.tile([128, TT], BF16, tag="xt")
                nc.sync.dma_start_transpose(
                    t[:], x[ts : ts + TT, k * 128 : (k + 1) * 128]
                )
                xt.append(t)
            return xt

        # ---- startup ----
        xt_queue = [emit_transposes(0)]
        fcsb = [load_fc_chunk(k) for k in range(4)]
        xt_queue.append(emit_transposes(1))

        fr_stage = stage.tile([1, DPAD], FP32, tag="frstage", bufs=1)
        nc.sync.dma_start(fr_stage[:], fc_row)
        f4 = const.tile([1, DPAD], FP32R, tag="f4")
        nc.vector.tensor_copy(f4[:], fr_stage[:])
        fcsb.append(f4)

        ones_st = stage.tile([1, 128], FP32, tag="ones_st", bufs=1)
        nc.vector.memset(ones_st[:], 1.0)
        ones = const.tile([1, 128], FP32R, tag="ones")
        nc.vector.tensor_copy(ones[:], ones_st[:])

        # per-batch bias const and c0, feature-major columns
        bct_sb = const.tile([128, 16 * B_LOC], FP32, tag="bct")
        nc.sync.dma_start(bct_sb[:], bct)
        c0_sb = const.tile([128, 4 * B_LOC], FP32, tag="c0")
        nc.sync.dma_start(c0_sb[:], c0t)

        # ---- main loop over 16 token tiles of 512 ----
        # (optionally repeated `reps` times inside one NEFF for timing)
        rep_ctx = tc.For_i(0, reps, 1) if reps > 1 else None
        if rep_ctx is not None:
            rep_ctx.__enter__()
        for tt in range(NT):
            b = tt // (T // TT)
            ts = tt * TT
            xt = xt_queue.pop(0)

            # prefetch + transpose two tiles ahead while this tile computes
            # (reps>1 wraps around so each For_i iteration is steady-state)
            if reps > 1:
                xt_queue.append(emit_transposes((tt + 2) % NT))
            elif tt + 2 < NT:
                xt_queue.append(emit_transposes(tt + 2))

            # gates + LSTM cell, per h-chunk k
            hn = []
            for k in range(4):
                gs = []
                for gi in range(4):
                    c = gi * 4 + k  # g-chunk index in [0, 16)
                    ps = gpp.tile([128, TT], FP32, tag="gp")
                    for kd in range(4):
                        nc.tensor.matmul(
                            ps[:],
                            wsb[kd][:, c * 128 : (c + 1) * 128],
                            xt[kd][:],
                            start=(kd == 0),
                            stop=False,
                        )
                    nc.tensor.matmul(
                        ps[:],
                        wsb[4][:, c * 128 : (c + 1) * 128],
                        xt[4][0:1, :],
                        start=False,
                        stop=True,
                    )
                    s = sigp.tile([128, TT], FP32, tag="sig")
                    nc.scalar.activation(
                        s[:],
                        ps[:],
                        gate_funcs[gi],
                        bias=bct_sb[:, c * B_LOC + b : c * B_LOC + b + 1],
                    )
                    gs.append(s)
                i_s, f_s, g_s, o_s = gs
                t1 = tmpp.tile([128, TT], FP32, tag="t1")
                nc.vector.tensor_mul(t1[:], i_s[:], g_s[:])
                t2 = tmpp.tile([128, TT], FP32, tag="t2")
                nc.vector.tensor_scalar_mul(
                    t2[:], f_s[:], c0_sb[:, k * B_LOC + b : k * B_LOC + b + 1]
                )
                cc = tmpp.tile([128, TT], FP32, tag="cc")
                nc.vector.tensor_add(cc[:], t1[:], t2[:])
                th = tmpp.tile([128, TT], FP32, tag="th")
                nc.scalar.activation(th[:], cc[:], AFT.Tanh)
                h = hp.tile([128, TT], FP32R, tag="h")
                nc.vector.tensor_mul(h[:], o_s[:], th[:])
                hn.append(h)

            # fc: out[tok, 513] per 128-token subtile, N split 256 + 258
            osb = outp.tile([128, NM, DPAD], FP32, tag="osb")
            for m in range(NM):
                msl = slice(m * 128, (m + 1) * 128)
                lhs5 = [
                    hn[0][:, msl],
                    hn[1][:, msl],
                    hn[2][:, msl],
                    hn[3][:, msl],
                    ones[:],
                ]
                pm = fmp.tile([128, NSPLIT[0]], FP32, tag="fm")
                for kd in range(5):
                    nc.tensor.matmul(
                        pm[:],
                        lhs5[kd],
                        fcsb[kd][:, 0 : NSPLIT[0]],
                        start=(kd == 0),
                        stop=(kd == 4),
                    )
                pc = fcp.tile([128, NSPLIT[1]], FP32, tag="fcc")
                for kd in range(5):
                    nc.tensor.matmul(
                        pc[:],
                        lhs5[kd],
                        fcsb[kd][:, NSPLIT[0] : DPAD],
                        start=(kd == 0),
                        stop=(kd == 4),
                    )
                nc.scalar.copy(osb[:, m, 0 : NSPLIT[0]], pm[:])
                nc.scalar.copy(osb[:, m, NSPLIT[0] : DPAD], pc[:])
                nc.sync.dma_start(
                    out[ts + m * 128 : ts + (m + 1) * 128, :], osb[:, m, 0:D]
                )
        if rep_ctx is not None:
            rep_ctx.__exit__(None, None, None)

    nc.compile()
    return nc


_NC_CACHE = []


def get_nc():
    if not _NC_CACHE:
        _NC_CACHE.append(build_nc())
    return _NC_CACHE[0]


def make_in_maps(decoder_inputs, h0, c0, W_ih, W_hh, b_ih, b_hh, fc_w, fc_b):
    di = np.asarray(decoder_inputs, dtype=np.float32)
    h0 = np.asarray(h0, dtype=np.float32)[0]  # [64, 512]
    c0 = np.asarray(c0, dtype=np.float32)[0]
    W_ih = np.asarray(W_ih, dtype=np.float32)
    W_hh = np.asarray(W_hh, dtype=np.float32)
    b_ih = np.asarray(b_ih, dtype=np.float32)
    b_hh = np.asarray(b_hh, dtype=np.float32)
    fc_w = np.asarray(fc_w, dtype=np.float32)
    fc_b = np.asarray(fc_b, dtype=np.float32)

    bc = h0 @ W_hh.T + b_ih + b_hh  # [64, 2048]
    wih_tp = np.ascontiguousarray(W_ih.T[0:512]).astype(ml_dtypes.bfloat16)
    wih_row_a = np.ascontiguousarray(W_ih.T[512:513]).astype(ml_dtypes.bfloat16)
    fc_pad = np.zeros((H, DPAD), dtype=np.float32)
    fc_pad[:, 0:D] = fc_w.T
    fc_row_a = np.zeros((1, DPAD), dtype=np.float32)
    fc_row_a[0, 0:D] = fc_b

    # x: bf16 cast + zero-pad to 640 cols (transpose windows of 128)
    x_pad = np.zeros((B * T, DX), dtype=ml_dtypes.bfloat16)
    x_pad[:, 0:D] = di.reshape(B * T, D).astype(ml_dtypes.bfloat16)

    in_maps = []
    for core in range(N_CORES):
        bs = core * B_LOC
        xc = x_pad[bs * T : (bs + B_LOC) * T]
        # bct[p, c*8+b] = bc[bs+b, c*128+p]
        bct = np.ascontiguousarray(
            bc[bs : bs + B_LOC]
            .reshape(B_LOC, 16, 128)
            .transpose(2, 1, 0)
            .reshape(128, -1)
        )
        c0c = np.ascontiguousarray(
            c0[bs : bs + B_LOC]
            .reshape(B_LOC, 4, 128)
            .transpose(2, 1, 0)
            .reshape(128, -1)
        )
        in_maps.append(
            {
                "x": xc,
                "wih_t": wih_tp,
                "wih_row": wih_row_a,
                "fc_rhs": fc_pad,
                "fc_row": fc_row_a,
                "bconst": bct,
                "c0t": c0c,
            }
        )
    return in_maps


def kernel(**inputs):
    in_maps = make_in_maps(**inputs)
    nc = get_nc()
    res = run_bass_kernel_spmd(nc, in_maps, core_ids=list(range(N_CORES)))
    out = np.concatenate([res.results[c]["out"] for c in range(N_CORES)], axis=0)
    return out.reshape(B, T, D)
